# revision 1
# baseline (speedup 1.0000x reference)
"""MultiHeadAttention Trainium2 kernel, 8-way tensor-parallel by head.

Problem: B=4, S=2048, D=1024, 16 heads, d_k=64 (nn_MultiHeadAttention_67585605370071).

Sharding: each core owns 2 heads (128 of the 1024 hidden dims):
  Wq/Wk/Wv column-sharded by head, Wo row-sharded; the 8 partial outputs
  are summed on the host (the row-shard reduction) and bo is added there.

Dataflow per core (matmuls bf16 / fp8-DoubleRow, f32 PSUM accumulation):
  - host passes x pre-transposed (xT [1024, 8192] bf16)
  - Qt/Kt/Vt = W @ xT + b in "feature-major" layout [128, tokens]
  - V rotated to natural [token, dv] layout via PE transposes, packed fp8 as
    [V_A | 1 | pad | V_B | 1 | pad] per 128-token tile; the ones column makes
    the AV matmul emit softmax denominators as row 64 of its PSUM tile
  - scoresT = Kt.T @ Qt per k-tile, two heads row-packed on the PE array
  - exp via ScalarE directly from PSUM (scale=1/8 fused), fp8 output
  - AV via fp8 DoubleRow matmuls (2 k-tiles per pass)
  - normalize via reciprocal + K=1 broadcast matmul + tensor_mul
  - out-proj with normalized attn tile stationary, WoT streaming
  - batches are software-pipelined: projection blocks for batch b+1 are
    interleaved with attention units of batch b so ScalarE (the exp
    bottleneck) never starves
"""
import numpy as np
import ml_dtypes

import concourse.bass as bass
import concourse.bacc as bacc
import concourse.mybir as mybir
import concourse.tile as tile
from concourse.bass_utils import run_bass_kernel_spmd

BF16 = mybir.dt.bfloat16
F32 = mybir.dt.float32
F8 = mybir.dt.float8e4
bf16 = ml_dtypes.bfloat16

B, S, D = 4, 2048, 1024
NT = B * S            # 8192 tokens
N_CORES = 8
KT_PER_B = S // 128   # 16 k-tiles per batch
QB_PER_B = S // 512   # 4 query blocks per batch
TB_PER_B = S // 512   # 4 token blocks per batch (projection)

# fp8-e4m3 attention-weights path: expT and V quantized to fp8 so the AV
# matmul can use DoubleRow (2x PE throughput). Measured end-to-end rel err
# ~1.5e-2 (vs ~2.4e-3 all-bf16).
FP8_AV = True
VG = 144 if FP8_AV else 130     # vaug per-k-tile column group
VDT = F8 if FP8_AV else BF16
HOFF = 72 if FP8_AV else 65     # head B column offset inside a group

_COMPILED = None


def _build():
    nc = bacc.Bacc("TRN2", target_bir_lowering=False, debug=False,
                   num_devices=N_CORES)

    xt_p = nc.declare_dram_parameter("xt", [D, NT], BF16, isOutput=False)
    wq_p = nc.declare_dram_parameter("wq", [128, D], BF16, isOutput=False)
    wk_p = nc.declare_dram_parameter("wk", [128, D], BF16, isOutput=False)
    wv_p = nc.declare_dram_parameter("wv", [128, D], BF16, isOutput=False)
    wo_p = nc.declare_dram_parameter("wo", [128, D], BF16, isOutput=False)
    bq_p = nc.declare_dram_parameter("bq", [128, 1], F32, isOutput=False)
    bk_p = nc.declare_dram_parameter("bk", [128, 1], F32, isOutput=False)
    bv_p = nc.declare_dram_parameter("bv", [128, 1], F32, isOutput=False)
    id_p = nc.declare_dram_parameter("ident", [128, 128], BF16, isOutput=False)
    out_p = nc.declare_dram_parameter("out", [NT, D], BF16, isOutput=True)

    with tile.TileContext(nc) as tc:
        with (
            tc.tile_pool(name="consts", bufs=1) as consts,
            tc.tile_pool(name="xts", bufs=3) as xts_pool,
            tc.tile_pool(name="qkv", bufs=2) as qkv_pool,
            tc.tile_pool(name="expp", bufs=4) as exp_pool,
            tc.tile_pool(name="attnp", bufs=3) as attn_pool,
            tc.tile_pool(name="small", bufs=3) as small,
            tc.tile_pool(name="outp", bufs=3) as out_pool,
            tc.tile_pool(name="ps_big", bufs=2, space=bass.MemorySpace.PSUM) as ps_big,
            tc.tile_pool(name="ps_av", bufs=2, space=bass.MemorySpace.PSUM) as ps_av,
            tc.tile_pool(name="ps_bc", bufs=1, space=bass.MemorySpace.PSUM) as ps_bc,
            tc.tile_pool(name="ps_proj", bufs=1, space=bass.MemorySpace.PSUM) as ps_proj,
        ):
            wq = consts.tile([128, D], BF16, name="wq")
            wk = consts.tile([128, D], BF16, name="wk")
            wv = consts.tile([128, D], BF16, name="wv")
            wo = consts.tile([128, D], BF16, name="wo")
            bq = consts.tile([128, 1], F32, name="bq")
            bk = consts.tile([128, 1], F32, name="bk")
            bv = consts.tile([128, 1], F32, name="bv")
            ident = consts.tile([128, 128], BF16, name="ident")
            ones64 = consts.tile([1, 64], F32, name="ones64")
            nc.sync.dma_start(wq[:], wq_p[:])
            nc.sync.dma_start(wk[:], wk_p[:])
            nc.sync.dma_start(wv[:], wv_p[:])
            nc.sync.dma_start(wo[:], wo_p[:])
            nc.sync.dma_start(bq[:], bq_p[:])
            nc.sync.dma_start(bk[:], bk_p[:])
            nc.sync.dma_start(bv[:], bv_p[:])
            nc.sync.dma_start(ident[:], id_p[:])
            nc.vector.memset(ones64[:], 1.0)

            # per-batch, per-block persistent tiles, allocated lazily
            qt = {}          # qt[b][tb] -> [128, 512]
            kt = {}          # kt[b][tb] -> [128, 512] (4 k-tiles each)
            vts = {}         # vts[b][tb] -> [128, 512]
            vaug = {}        # vaug[b][tb] -> [128, 4*VG] fp8
            xts_t = {}       # prefetched xT stacks

            def _alloc_batch(b):
                qt[b] = [qkv_pool.tile([128, 512], BF16, tag=f"qt{t}",
                                       name=f"qt{b}_{t}") for t in range(4)]
                kt[b] = [qkv_pool.tile([128, 512], BF16, tag=f"kt{t}",
                                       name=f"kt{b}_{t}") for t in range(4)]
                vts[b] = [qkv_pool.tile([128, 512], BF16, tag=f"vts{t}",
                                        name=f"vts{b}_{t}") for t in range(4)]
                vaug[b] = []
                for t in range(4):
                    va = qkv_pool.tile([128, 4 * VG], VDT, tag=f"vaug{t}",
                                       name=f"vaug{b}_{t}")
                    va3 = va.rearrange("p (k c) -> p k c", c=VG)
                    nc.vector.memset(va3[:, :, 64:65], 1.0)
                    nc.vector.memset(va3[:, :, HOFF + 64:HOFF + 65], 1.0)
                    vaug[b].append(va)

            def emit_proj_dma(b, tb):
                """Prefetch the 512-token xT stack for block (b, tb)."""
                if b not in qt:
                    _alloc_batch(b)
                tok0 = b * S + tb * 512
                xts = xts_pool.tile([128, 8 * 512], BF16, tag="xts",
                                    name=f"xts{b}_{tb}")
                src3 = xt_p.rearrange("(kd p) t -> p kd t", p=128)
                dst3 = xts.rearrange("p (kd t) -> p kd t", t=512)
                nc.sync.dma_start(dst3[:, :, :], src3[:, :, tok0:tok0 + 512])
                xts_t[(b, tb)] = xts

            def emit_proj_compute(b, tb):
                """Project one 512-token block of batch b into qt/kt/vts and
                rotate its V into vaug."""
                for clo in proj_closures(b, tb):
                    clo()

            def proj_closures(b, tb):
                """The projection block as a list of small closures so its PE
                work can be interleaved between attention score chunks."""
                xts = xts_t.pop((b, tb))
                clos = []
                for w_sb, b_sb, dst in ((wq, bq, qt[b][tb]),
                                        (wk, bk, kt[b][tb]),
                                        (wv, bv, vts[b][tb])):
                    pp = ps_proj.tile([128, 512], F32, tag="proj",
                                      name=f"pp{b}_{tb}_{dst.name}")

                    def mm2(pp=pp, w_sb=w_sb, xts=xts, kd0=0):
                        for kd in (kd0, kd0 + 1):
                            nc.tensor.matmul(
                                pp[:], w_sb[:, 128 * kd:128 * kd + 128],
                                xts[:, 512 * kd:512 * kd + 512],
                                start=(kd == 0), stop=(kd == 7))
                    for kd0 in (0, 2, 4, 6):
                        clos.append(
                            lambda pp=pp, w_sb=w_sb, xts=xts, kd0=kd0:
                            mm2(pp, w_sb, xts, kd0))
                    clos.append(lambda dst=dst, pp=pp, b_sb=b_sb:
                                nc.vector.tensor_scalar_add(dst[:], pp[:],
                                                            b_sb[:]))

                def rot(ti):
                    tp = ps_bc.tile([128, 128], BF16, tag="bc",
                                    name=f"tp{b}_{tb}_{ti}")
                    nc.tensor.transpose(
                        tp[:], vts[b][tb][:, 128 * ti:128 * ti + 128], ident[:])
                    nc.vector.tensor_copy(
                        vaug[b][tb][:, VG * ti:VG * ti + 64], tp[:, 0:64])
                    nc.vector.tensor_copy(
                        vaug[b][tb][:, VG * ti + HOFF:VG * ti + HOFF + 64],
                        tp[:, 64:128])
                for ti in range(4):
                    clos.append(lambda ti=ti: rot(ti))
                return clos

            def outproj_closures(b, qb, attn):
                clos = []

                def op_half(ti, j, ob):
                    po = ps_big.tile([128, 512], F32, tag="big",
                                     name=f"po{b}_{qb}_{ti}_{j}")
                    nc.tensor.matmul(po[:],
                                     attn[:, 128 * ti:128 * ti + 128],
                                     wo[:, 512 * j:512 * j + 512],
                                     start=True, stop=True)
                    nc.vector.tensor_copy(ob[:, 512 * j:512 * j + 512], po[:])
                    if j == 1:
                        row0 = b * S + 512 * qb + 128 * ti
                        nc.sync.dma_start(out_p[row0:row0 + 128, :], ob[:])
                for ti in range(4):
                    ob = out_pool.tile([128, 1024], BF16, tag="ob",
                                       name=f"ob{b}_{qb}_{ti}")
                    for j in range(2):
                        clos.append(lambda ti=ti, j=j, ob=ob: op_half(ti, j, ob))
                return clos

            def emit_attention_unit(b, qb, bg):
                """scores+exp+AV+norm for one 512-query block.

                Per 2-k-tile chunk: scores matmuls -> one exp ACT -> one
                fp8-DoubleRow AV matmul, so the AV accumulation trails the
                exp stream chunk-by-chunk. Background closures (projection
                blocks for the next batch, out-proj of the previous unit)
                are drained between chunks to keep PE busy without ever
                putting a long contiguous block ahead of the next scores."""
                qsl = qt[b][qb]
                av = [ps_av.tile([65, 512], F32, tag="av",
                                 name=f"av{b}_{qb}_{h}") for h in range(2)]
                for c in range(KT_PER_B // 2):
                    tb_of_c = c // 2        # source projection block
                    cc = c % 2              # k-tile pair within that block
                    va3 = vaug[b][tb_of_c].rearrange("p (k c) -> p k c", c=VG)
                    for h in range(2):
                        sp = ps_big.tile([128, 1024], F32, tag="big",
                                         name=f"sp{b}_{qb}_{c}_{h}")
                        for j in range(2):
                            kt_loc = 2 * cc + j
                            nc.tensor.matmul(
                                sp[:, 512 * j:512 * j + 512],
                                kt[b][tb_of_c][64 * h:64 * h + 64,
                                               128 * kt_loc:128 * kt_loc + 128],
                                qsl[64 * h:64 * h + 64, :],
                                start=True, stop=True)
                        ec = exp_pool.tile([128, 1024], VDT, tag=f"expt{h}",
                                           name=f"ec{b}_{qb}_{c}_{h}")
                        nc.scalar.activation(
                            ec[:], sp[:],
                            mybir.ActivationFunctionType.Exp, scale=0.125)
                        ec3 = ec.rearrange("p (k q) -> p k q", q=512)
                        if FP8_AV:
                            nc.tensor.matmul(
                                av[h][:],
                                va3[:, 2 * cc:2 * cc + 2, HOFF * h:HOFF * h + 65],
                                ec3[:, :, :],
                                start=(c == 0), stop=(c == KT_PER_B // 2 - 1),
                                perf_mode=mybir.MatmulPerfMode.DoubleRow)
                        else:
                            for j in range(2):
                                nc.tensor.matmul(
                                    av[h][:],
                                    vaug[b][tb_of_c][
                                        :, VG * (2 * cc + j) + HOFF * h:
                                        VG * (2 * cc + j) + HOFF * h + 65],
                                    ec[:, 512 * j:512 * j + 512],
                                    start=(c == 0 and j == 0),
                                    stop=(c == KT_PER_B // 2 - 1 and j == 1))
                        for _ in range(2):
                            if bg:
                                bg.popleft()()
                attn = attn_pool.tile([128, 512], BF16, tag="attn",
                                      name=f"attn{b}_{qb}")
                for h in range(2):
                    rrow = small.tile([1, 512], F32, tag="rrow",
                                      name=f"rr{b}_{qb}_{h}")
                    nc.vector.reciprocal(rrow[:], av[h][64:65, :])
                    bc = ps_bc.tile([64, 512], F32, tag="bc",
                                    name=f"bc{b}_{qb}_{h}")
                    nc.tensor.matmul(bc[:], ones64[:], rrow[:],
                                     start=True, stop=True)
                    bc_sb = small.tile([64, 512], F32, tag="bc_sb",
                                       name=f"bs{b}_{qb}_{h}")
                    nc.vector.tensor_copy(bc_sb[:], bc[:])
                    nc.vector.tensor_mul(attn[64 * h:64 * h + 64, :],
                                         av[h][0:64, :], bc_sb[:])
                return attn

            # software pipeline: full proj for b=0 (DMAs prefetched first),
            # then attention(b) units with proj blocks of b+1 and the
            # previous unit's out-proj spread between score chunks
            from collections import deque
            bg = deque()
            for tb in range(TB_PER_B):
                emit_proj_dma(0, tb)
            for tb in range(TB_PER_B):
                emit_proj_compute(0, tb)
            for b in range(B):
                for qb in range(QB_PER_B):
                    if b + 1 < B:
                        emit_proj_dma(b + 1, qb)
                        bg.extend(proj_closures(b + 1, qb))
                    attn = emit_attention_unit(b, qb, bg)
                    bg.extend(outproj_closures(b, qb, attn))
            while bg:
                bg.popleft()()

    nc.compile()
    return nc


def _get_compiled():
    global _COMPILED
    if _COMPILED is None:
        _COMPILED = _build()
    return _COMPILED


def _prep_inputs(x, Wq, bq, Wk, bk, Wv, bv, Wo, bo):
    xt = np.ascontiguousarray(
        np.asarray(x, dtype=np.float32).reshape(NT, D).T).astype(bf16)
    ident = np.eye(128, dtype=bf16)

    def pack_w(Wc):  # [128 out, 1024 in] -> k-tile packed [128, 1024]
        wt = np.asarray(Wc, dtype=np.float32).T  # [1024 in, 128 out]
        return np.ascontiguousarray(
            wt.reshape(8, 128, 128).transpose(1, 0, 2).reshape(128, D)).astype(bf16)

    in_maps = []
    for c in range(N_CORES):
        sl = slice(128 * c, 128 * c + 128)
        in_maps.append({
            "xt": xt,
            "wq": pack_w(np.asarray(Wq)[sl]),
            "wk": pack_w(np.asarray(Wk)[sl]),
            "wv": pack_w(np.asarray(Wv)[sl]),
            "wo": np.ascontiguousarray(
                np.asarray(Wo, dtype=np.float32)[:, sl].T).astype(bf16),
            "bq": np.asarray(bq, dtype=np.float32)[sl].reshape(128, 1),
            "bk": np.asarray(bk, dtype=np.float32)[sl].reshape(128, 1),
            "bv": np.asarray(bv, dtype=np.float32)[sl].reshape(128, 1),
            "ident": ident,
        })
    return in_maps


def kernel(x, Wq, bq, Wk, bk, Wv, bv, Wo, bo):
    nc = _get_compiled()
    in_maps = _prep_inputs(x, Wq, bq, Wk, bk, Wv, bv, Wo, bo)
    res = run_bass_kernel_spmd(nc, in_maps, core_ids=list(range(N_CORES)))
    acc = np.zeros((NT, D), dtype=np.float32)
    for c in range(N_CORES):
        acc += res.results[c]["out"].astype(np.float32)
    acc += np.asarray(bo, dtype=np.float32)[None, :]
    return acc.reshape(B, S, D)

